# revision 16
# baseline (speedup 1.0000x reference)
"""MoE AlltoAllTokenDispatcher kernel for TRN2 (8 NeuronCores).

The reference dispatcher's gather (tokens[argsort(idx)//k]) followed by
scatter-add at the same argsort permutation is an exact identity on slot
order: unpermuted[s] == tokens[s // k] for every slot s, independent of the
routing indices. The whole module therefore reduces to

    out[i] = tokens[i] * (probs[i, 0] + probs[i, 1])

which is a pure memory-bound row-scaling. We shard the token dimension
across the 8 cores (data-parallel, per the sharding hint's token-dim
sharding; no all-to-all is needed since expert compute is identity).

Per-core layout: token tile i holds tokens {16p + i} on partition p
(stride-16 rows). With that tiling, probs loads as ONE fully contiguous
[128, 32] tile (128 B/partition) and a single strided DVE pair-add yields
every tile's per-partition scale column — no transpose, no tiny-descriptor
gather clogging the DMA rings.

Loads ride the sync HWDGE ring and stores the scalar HWDGE ring, so a
store waiting on compute never blocks a later load's dispatch.
"""

import numpy as np

import concourse.bass as bass
import concourse.tile as tile
from concourse import bacc, mybir
from concourse.bass_utils import run_bass_kernel_spmd

N_TOKENS = 16384
HIDDEN = 4096
TOP_K = 2
N_CORES = 8
TOK_PER_CORE = N_TOKENS // N_CORES  # 2048
P = 128
N_TILES = TOK_PER_CORE // P  # 16
import os as _os

N_BUFS = int(_os.environ.get("KERNEL_BUFS", "6"))

_nc_cache = None


def _work_items():
    """(tile_idx, col_start, ncols) per pipeline step. Edge tiles are split
    along hidden (quarters at the very ends, then halves) so the pipeline
    ramps and drains with short dependency chains; middle tiles stay full
    for max DMA packet efficiency."""
    import os

    split_n = int(os.environ.get("KERNEL_EDGE_SPLITS", "2"))
    if os.environ.get("KERNEL_ALL_SPLIT", "0") == "1":
        splits = {i: 2 for i in range(N_TILES)}
    else:
        splits = {0: split_n, N_TILES - 1: split_n} if split_n > 1 else {}
    items = []
    for i in range(N_TILES):
        n = splits.get(i, 1)
        w = HIDDEN // n
        for h in range(n):
            items.append((i, h * w, w))
    return items


def _emit_probs_scales(nc, pt, st, probs):
    """pt[p, (j k)] <- probs[16p+j, k] (one contiguous DMA), then
    st[p, j] = pt[p, 2j] + pt[p, 2j+1] (one strided DVE add)."""
    nc.sync.dma_start(
        out=pt[:],
        in_=probs.rearrange("(p j) k -> p (j k)", j=N_TILES),
    )
    pt3 = pt[:].rearrange("p (j k) -> p j k", k=TOP_K)
    nc.vector.tensor_add(
        st[:].rearrange("p (j o) -> p j o", o=1),
        pt3[:, :, 0:1],
        pt3[:, :, 1:2],
    )


def _tok_view(tokens):
    # tile i, partition p  <->  token row 16p + i
    return tokens.rearrange("(p n) m -> n p m", n=N_TILES)


def _build_nc_tile():
    nc = bacc.Bacc(
        "TRN2", target_bir_lowering=False, debug=False, num_devices=N_CORES
    )
    tokens = nc.dram_tensor(
        "tokens", [TOK_PER_CORE, HIDDEN], mybir.dt.float32, kind="ExternalInput"
    ).ap()
    probs = nc.dram_tensor(
        "probs", [TOK_PER_CORE, TOP_K], mybir.dt.float32, kind="ExternalInput"
    ).ap()
    out = nc.dram_tensor(
        "out", [TOK_PER_CORE, HIDDEN], mybir.dt.float32, kind="ExternalOutput"
    ).ap()
    tok_t = _tok_view(tokens)
    out_t = _tok_view(out)

    import os

    separate_out = os.environ.get("KERNEL_SEPARATE_OUT", "0") == "1"
    out_bufs = int(os.environ.get("KERNEL_OUT_BUFS", "5"))
    with tile.TileContext(nc) as tc:
        with (
            tc.tile_pool(name="tok", bufs=N_BUFS) as tok_pool,
            tc.tile_pool(name="ot", bufs=out_bufs) as out_pool,
            tc.tile_pool(name="pr", bufs=1) as pr_pool,
            tc.tile_pool(name="sc", bufs=1) as sc_pool,
        ):
            pt = pr_pool.tile([P, N_TILES * TOP_K], mybir.dt.float32)
            st = sc_pool.tile([P, N_TILES], mybir.dt.float32)
            _emit_probs_scales(nc, pt, st, probs)

            for i, c0, ncols in _work_items():
                tt = tok_pool.tile([P, ncols], mybir.dt.float32, tag="tok")
                nc.sync.dma_start(
                    out=tt[:, :ncols], in_=tok_t[i, :, c0 : c0 + ncols]
                )
                if separate_out:
                    ot = out_pool.tile([P, ncols], mybir.dt.float32, tag="ot")
                else:
                    ot = tt
                nc.vector.tensor_scalar_mul(
                    ot[:, :ncols], tt[:, :ncols], st[:, i : i + 1]
                )
                nc.scalar.dma_start(
                    out=out_t[i, :, c0 : c0 + ncols], in_=ot[:, :ncols]
                )
    nc.compile()
    return nc


def _build_nc_raw():
    nc = bass.Bass("TRN2", target_bir_lowering=False, debug=False)
    tokens = nc.dram_tensor(
        "tokens", [TOK_PER_CORE, HIDDEN], mybir.dt.float32, kind="ExternalInput"
    )
    probs = nc.dram_tensor(
        "probs", [TOK_PER_CORE, TOP_K], mybir.dt.float32, kind="ExternalInput"
    )
    out = nc.dram_tensor(
        "out", [TOK_PER_CORE, HIDDEN], mybir.dt.float32, kind="ExternalOutput"
    )
    tok_t = _tok_view(tokens)
    out_t = _tok_view(out)

    items = _work_items()
    n_items = len(items)

    with (
        nc.sbuf_tensor("buf", [P, N_BUFS * HIDDEN], mybir.dt.float32) as buf,
        nc.sbuf_tensor("pt", [P, N_TILES * TOP_K], mybir.dt.float32) as pt,
        nc.sbuf_tensor("st", [P, N_TILES], mybir.dt.float32) as st,
        nc.semaphore("probs_sem") as probs_sem,
        nc.semaphore("load_sem") as load_sem,
        nc.semaphore("comp_sem") as comp_sem,
        nc.semaphore("store_sem") as store_sem,
        nc.Block() as block,
    ):

        def slot(j, ncols):
            base = (j % N_BUFS) * HIDDEN
            return buf[:, base : base + ncols]

        @block.sync
        def _(sync: bass.BassEngine):
            sync.dma_start(
                out=pt[:],
                in_=probs.rearrange("(p j) k -> p (j k)", j=N_TILES),
            ).then_inc(probs_sem, 16)
            for j, (i, c0, ncols) in enumerate(items):
                if j >= N_BUFS:
                    sync.wait_ge(store_sem, 16 * (j - N_BUFS + 1))
                sync.dma_start(
                    out=slot(j, ncols), in_=tok_t[i, :, c0 : c0 + ncols]
                ).then_inc(load_sem, 16)

        @block.vector
        def _(vector: bass.BassEngine):
            vector.wait_ge(probs_sem, 16)
            pt3 = pt[:].rearrange("p (j k) -> p j k", k=TOP_K)
            vector.tensor_add(
                st[:].rearrange("p (j o) -> p j o", o=1),
                pt3[:, :, 0:1],
                pt3[:, :, 1:2],
            )
            for j, (i, c0, ncols) in enumerate(items):
                vector.wait_ge(load_sem, 16 * (j + 1))
                vector.tensor_scalar_mul(
                    slot(j, ncols), slot(j, ncols), st[:, i : i + 1]
                ).then_inc(comp_sem, 1)

        @block.scalar
        def _(scalar: bass.BassEngine):
            for j, (i, c0, ncols) in enumerate(items):
                scalar.wait_ge(comp_sem, j + 1)
                scalar.dma_start(
                    out=out_t[i, :, c0 : c0 + ncols], in_=slot(j, ncols)
                ).then_inc(store_sem, 16)
            scalar.wait_ge(store_sem, 16 * n_items)

    return nc


def _build():
    import os

    if os.environ.get("KERNEL_VARIANT", "tile") == "raw":
        return _build_nc_raw()
    return _build_nc_tile()


def kernel(tokens, probs, indices=None, **_unused):
    global _nc_cache
    tokens = np.ascontiguousarray(np.asarray(tokens, dtype=np.float32))
    probs = np.ascontiguousarray(np.asarray(probs, dtype=np.float32))
    assert tokens.shape == (N_TOKENS, HIDDEN)
    assert probs.shape == (N_TOKENS, TOP_K)

    if _nc_cache is None:
        _nc_cache = _build()
    nc = _nc_cache

    in_maps = [
        {
            "tokens": tokens[c * TOK_PER_CORE : (c + 1) * TOK_PER_CORE],
            "probs": probs[c * TOK_PER_CORE : (c + 1) * TOK_PER_CORE],
        }
        for c in range(N_CORES)
    ]
    res = run_bass_kernel_spmd(nc, in_maps, core_ids=list(range(N_CORES)))
    return np.concatenate([res.results[c]["out"] for c in range(N_CORES)], axis=0)


# revision 17
# speedup vs baseline: 1.1450x; 1.1450x over previous
"""MoE AlltoAllTokenDispatcher kernel for TRN2 (8 NeuronCores).

The reference dispatcher's gather (tokens[argsort(idx)//k]) followed by
scatter-add at the same argsort permutation is an exact identity on slot
order: unpermuted[s] == tokens[s // k] for every slot s, independent of the
routing indices. The whole module therefore reduces to

    out[i] = tokens[i] * (probs[i, 0] + probs[i, 1])

a pure memory-bound row-scaling (the memory roofline is read 256 MB +
write 256 MB). Tokens are sharded across the 8 cores on the token dim
(data-parallel per the sharding hint; no all-to-all is needed since the
expert compute between dispatch and combine is identity).

Per-core kernel (Tile framework):
  - Token tile i holds tokens {16p + i} on partition p (stride-16 rows).
    With that tiling probs loads as ONE fully contiguous [128, 32] tile
    (128 B/partition) and a single strided DVE pair-add produces every
    tile's per-partition scale column — no transpose and no tiny-descriptor
    gather clogging the DMA rings (an (n p)-ordered gather would emit 2048
    8-byte descriptors and stall each SDMA engine ~15 us).
  - Loads ride the sync HWDGE ring, stores the scalar HWDGE ring, so a
    store waiting on compute never blocks a later load's dispatch.
  - First and last tiles are split in half along hidden: shorter dependency
    chains at ramp/drain. (Empirically load-bearing: without the splits the
    slot-recycle loop latency exceeds bufs x the 9.7 us/tile bandwidth
    period and the dispatch cadence drifts, costing ~36 us.)

Measured on 8 concurrent trn2 cores: ~169.6 us/core (~405 GB/s of the
~425 GB/s 16-SDMA-engine ceiling; the remainder is the fixed ~7 us
framework preamble plus ramp/drain).
"""

import numpy as np

import concourse.tile as tile
from concourse import bacc, mybir
from concourse.bass_utils import run_bass_kernel_spmd

N_TOKENS = 16384
HIDDEN = 4096
TOP_K = 2
N_CORES = 8
TOK_PER_CORE = N_TOKENS // N_CORES  # 2048
P = 128
N_TILES = TOK_PER_CORE // P  # 16
N_BUFS = 6

_nc_cache = None


def _work_items():
    """(tile_idx, col_start, ncols) per pipeline step; first and last row
    tiles are split in half along hidden."""
    items = []
    h2 = HIDDEN // 2
    for i in range(N_TILES):
        if i in (0, N_TILES - 1):
            items.append((i, 0, h2))
            items.append((i, h2, h2))
        else:
            items.append((i, 0, HIDDEN))
    return items


def _build_nc():
    nc = bacc.Bacc(
        "TRN2", target_bir_lowering=False, debug=False, num_devices=N_CORES
    )
    tokens = nc.dram_tensor(
        "tokens", [TOK_PER_CORE, HIDDEN], mybir.dt.float32, kind="ExternalInput"
    ).ap()
    probs = nc.dram_tensor(
        "probs", [TOK_PER_CORE, TOP_K], mybir.dt.float32, kind="ExternalInput"
    ).ap()
    out = nc.dram_tensor(
        "out", [TOK_PER_CORE, HIDDEN], mybir.dt.float32, kind="ExternalOutput"
    ).ap()
    # tile i, partition p  <->  token row 16p + i
    tok_t = tokens.rearrange("(p n) m -> n p m", n=N_TILES)
    out_t = out.rearrange("(p n) m -> n p m", n=N_TILES)

    with tile.TileContext(nc) as tc:
        with (
            tc.tile_pool(name="tok", bufs=N_BUFS) as tok_pool,
            tc.tile_pool(name="pr", bufs=1) as pr_pool,
            tc.tile_pool(name="sc", bufs=1) as sc_pool,
        ):
            # pt[p, (j k)] <- probs[16p+j, k]: one contiguous DMA, then
            # st[p, j] = pt[p, 2j] + pt[p, 2j+1]: one strided DVE add.
            pt = pr_pool.tile([P, N_TILES * TOP_K], mybir.dt.float32)
            st = sc_pool.tile([P, N_TILES], mybir.dt.float32)
            nc.sync.dma_start(
                out=pt[:],
                in_=probs.rearrange("(p j) k -> p (j k)", j=N_TILES),
            )
            pt3 = pt[:].rearrange("p (j k) -> p j k", k=TOP_K)
            nc.vector.tensor_add(
                st[:].rearrange("p (j o) -> p j o", o=1),
                pt3[:, :, 0:1],
                pt3[:, :, 1:2],
            )

            for i, c0, ncols in _work_items():
                tt = tok_pool.tile([P, ncols], mybir.dt.float32, tag="tok")
                nc.sync.dma_start(
                    out=tt[:, :ncols], in_=tok_t[i, :, c0 : c0 + ncols]
                )
                nc.vector.tensor_scalar_mul(
                    tt[:, :ncols], tt[:, :ncols], st[:, i : i + 1]
                )
                nc.scalar.dma_start(
                    out=out_t[i, :, c0 : c0 + ncols], in_=tt[:, :ncols]
                )
    nc.compile()
    return nc


def kernel(tokens, probs, indices=None, **_unused):
    global _nc_cache
    tokens = np.ascontiguousarray(np.asarray(tokens, dtype=np.float32))
    probs = np.ascontiguousarray(np.asarray(probs, dtype=np.float32))
    assert tokens.shape == (N_TOKENS, HIDDEN), tokens.shape
    assert probs.shape == (N_TOKENS, TOP_K), probs.shape

    if _nc_cache is None:
        _nc_cache = _build_nc()

    in_maps = [
        {
            "tokens": tokens[c * TOK_PER_CORE : (c + 1) * TOK_PER_CORE],
            "probs": probs[c * TOK_PER_CORE : (c + 1) * TOK_PER_CORE],
        }
        for c in range(N_CORES)
    ]
    res = run_bass_kernel_spmd(
        _nc_cache, in_maps, core_ids=list(range(N_CORES))
    )
    return np.concatenate([res.results[c]["out"] for c in range(N_CORES)], axis=0)


# revision 18
# speedup vs baseline: 1.1480x; 1.0026x over previous
"""MoE AlltoAllTokenDispatcher kernel for TRN2 (8 NeuronCores).

The reference dispatcher's gather (tokens[argsort(idx)//k]) followed by
scatter-add at the same argsort permutation is an exact identity on slot
order: unpermuted[s] == tokens[s // k] for every slot s, independent of the
routing indices. The whole module therefore reduces to

    out[i] = tokens[i] * (probs[i, 0] + probs[i, 1])

a pure memory-bound row-scaling (the memory roofline is read 256 MB +
write 256 MB). Tokens are sharded across the 8 cores on the token dim
(data-parallel per the sharding hint; no all-to-all is needed since the
expert compute between dispatch and combine is identity).

Per-core kernel (Tile framework):
  - Token tile i holds tokens {16p + i} on partition p (stride-16 rows).
    With that tiling probs loads as ONE fully contiguous [128, 32] tile
    (128 B/partition) and a single strided DVE pair-add produces every
    tile's per-partition scale column — no transpose and no tiny-descriptor
    gather clogging the DMA rings (an (n p)-ordered gather would emit 2048
    8-byte descriptors and stall each SDMA engine ~15 us).
  - Loads ride the sync HWDGE ring, stores the scalar HWDGE ring, so a
    store waiting on compute never blocks a later load's dispatch.
  - First and last tiles are split in half along hidden: shorter dependency
    chains at ramp/drain. (Empirically load-bearing: without the splits the
    slot-recycle loop latency exceeds bufs x the 9.7 us/tile bandwidth
    period and the dispatch cadence drifts, costing ~36 us.)

Measured on 8 concurrent trn2 cores: ~169.6 us/core (~405 GB/s of the
~425 GB/s 16-SDMA-engine ceiling; the remainder is the fixed ~7 us
framework preamble plus ramp/drain).
"""

import numpy as np

import concourse.tile as tile
from concourse import bacc, mybir
from concourse.bass_utils import run_bass_kernel_spmd

N_TOKENS = 16384
HIDDEN = 4096
TOP_K = 2
N_CORES = 8
TOK_PER_CORE = N_TOKENS // N_CORES  # 2048
P = 128
N_TILES = TOK_PER_CORE // P  # 16
N_BUFS = 6

_nc_cache = None


def _work_items():
    """(tile_idx, col_start, ncols) per pipeline step; first and last row
    tiles are split in half along hidden."""
    items = []
    h2 = HIDDEN // 2
    for i in range(N_TILES):
        if i in (N_TILES - 1,):
            items.append((i, 0, h2))
            items.append((i, h2, h2))
        else:
            items.append((i, 0, HIDDEN))
    return items


def _build_nc():
    nc = bacc.Bacc(
        "TRN2", target_bir_lowering=False, debug=False, num_devices=N_CORES
    )
    tokens = nc.dram_tensor(
        "tokens", [TOK_PER_CORE, HIDDEN], mybir.dt.float32, kind="ExternalInput"
    ).ap()
    probs = nc.dram_tensor(
        "probs", [TOK_PER_CORE, TOP_K], mybir.dt.float32, kind="ExternalInput"
    ).ap()
    out = nc.dram_tensor(
        "out", [TOK_PER_CORE, HIDDEN], mybir.dt.float32, kind="ExternalOutput"
    ).ap()
    # tile i, partition p  <->  token row 16p + i
    tok_t = tokens.rearrange("(p n) m -> n p m", n=N_TILES)
    out_t = out.rearrange("(p n) m -> n p m", n=N_TILES)

    with tile.TileContext(nc) as tc:
        with (
            tc.tile_pool(name="tok", bufs=N_BUFS) as tok_pool,
            tc.tile_pool(name="pr", bufs=1) as pr_pool,
            tc.tile_pool(name="sc", bufs=1) as sc_pool,
        ):
            # pt[p, (j k)] <- probs[16p+j, k]: one contiguous DMA, then
            # st[p, j] = pt[p, 2j] + pt[p, 2j+1]: one strided DVE add.
            pt = pr_pool.tile([P, N_TILES * TOP_K], mybir.dt.float32)
            st = sc_pool.tile([P, N_TILES], mybir.dt.float32)
            nc.sync.dma_start(
                out=pt[:],
                in_=probs.rearrange("(p j) k -> p (j k)", j=N_TILES),
            )
            pt3 = pt[:].rearrange("p (j k) -> p j k", k=TOP_K)
            nc.vector.tensor_add(
                st[:].rearrange("p (j o) -> p j o", o=1),
                pt3[:, :, 0:1],
                pt3[:, :, 1:2],
            )

            for i, c0, ncols in _work_items():
                tt = tok_pool.tile([P, ncols], mybir.dt.float32, tag="tok")
                nc.sync.dma_start(
                    out=tt[:, :ncols], in_=tok_t[i, :, c0 : c0 + ncols]
                )
                nc.vector.tensor_scalar_mul(
                    tt[:, :ncols], tt[:, :ncols], st[:, i : i + 1]
                )
                nc.scalar.dma_start(
                    out=out_t[i, :, c0 : c0 + ncols], in_=tt[:, :ncols]
                )
    nc.compile()
    return nc


def kernel(tokens, probs, indices=None, **_unused):
    global _nc_cache
    tokens = np.ascontiguousarray(np.asarray(tokens, dtype=np.float32))
    probs = np.ascontiguousarray(np.asarray(probs, dtype=np.float32))
    assert tokens.shape == (N_TOKENS, HIDDEN), tokens.shape
    assert probs.shape == (N_TOKENS, TOP_K), probs.shape

    if _nc_cache is None:
        _nc_cache = _build_nc()

    in_maps = [
        {
            "tokens": tokens[c * TOK_PER_CORE : (c + 1) * TOK_PER_CORE],
            "probs": probs[c * TOK_PER_CORE : (c + 1) * TOK_PER_CORE],
        }
        for c in range(N_CORES)
    ]
    res = run_bass_kernel_spmd(
        _nc_cache, in_maps, core_ids=list(range(N_CORES))
    )
    return np.concatenate([res.results[c]["out"] for c in range(N_CORES)], axis=0)
